# revision 16
# baseline (speedup 1.0000x reference)
"""Trainium2 Bass kernel for nn_BalanceLabelAugmentation2 (topk_masking).

Math (reference, restructured):
  Z   = feat @ W.T            [N, 51]   (matmul is linear over the mixup!)
  lo  = feat_u @ W_o.T + b_o  [N_u, 51] -> pred=argmax, score=max softmax
  midw_i  = gm[pred_i] & (score_i > 0.5);  tailw_i = gt[pred_i] & (score_i > 0.3)
  For pair (copy c, unlabeled row i) with partner j = idx_c[i]:
    l    = 0.7*Z_o[j] + b + 0.3*Z_u[i]
    ce   = logsumexp(l) - sum(l * (0.7*onehot(label_j) + 0.3*onehot(pred_i)))
  out = sum(ce*w) / max(sum w, 1)

Distribution (8 cores, data-parallel rows):
  core r owns labeled rows [2048r, 2048(r+1)) and unlabeled rows likewise.
  Phase A: matmul labeled shard -> table row j = [L_j=0.7*Z_o[j]+b |
           0.7*onehot(label_j) | pad] (f32, 512B rows), AllGather the table.
  Phase B: matmul unlabeled shard (both heads) -> ZU=0.3*Z_u, onehot(pred),
           score, masks.
  Phase 3: dma_gather table rows for the core's 5*2048 pairs (640 rows/chunk,
           issued back-to-back on GpSimd right after the AllGather), fused
           DVE/ACT soft-CE with stable logsumexp, weighted accumulate.
  Final:   per-core [ce_sum, w_sum] -> AllGather -> each core computes scalar.

feat is cast to bf16 on the host (halves DMA, enables the HW xbar
DMA-transpose loads; ~1e-5 end-to-end effect on the reference inputs).
All post-matmul math is f32.

Engine placement: GpSimd = collectives + the 16 dma_gathers (desc-gen is the
phase-3 floor) + constant loads; labeled-shard transposed loads on Sync,
unlabeled on Scalar, all emitted up front so they stream ahead of compute.
"""

import numpy as np
import ml_dtypes

import concourse.bass as bass
import concourse.tile as tile
from concourse import bacc, mybir
from concourse.bass_utils import run_bass_kernel_spmd
from concourse.masks import make_identity

F32 = mybir.dt.float32
BF16 = mybir.dt.bfloat16
I16 = mybir.dt.int16
AF = mybir.ActivationFunctionType
ALU = mybir.AluOpType
AX = mybir.AxisListType


class Cfg:
    def __init__(self, n_o=16384, n_u=16384, d=1024, cores=8, rowt=512):
        self.n_o, self.n_u, self.d, self.cores, self.rowt = n_o, n_u, d, cores, rowt
        self.c = 51
        self.s = n_o // cores          # labeled rows per core
        self.u = n_u // cores          # unlabeled rows per core
        self.kc = d // 128             # contraction chunks
        self.lab_tiles = self.s // rowt
        self.unl_tiles = self.u // rowt
        self.cpt = rowt // 128         # 128-row chunks per tile
        self.lab_chunks = self.s // 128
        self.chunks = self.u // 128    # unlabeled 128-row chunks
        self.trow = 128                # table row f32 elems (512B; %256B for gather)
        assert self.s % rowt == 0 and self.u % rowt == 0 and d % 128 == 0


def _bc(tile_ap, offset_ap, pattern):
    """AP on tile_ap's tensor at offset_ap's offset with a custom free pattern."""
    return bass.AP(tensor=tile_ap.tensor, offset=offset_ap.offset,
                   ap=[tile_ap.ap[0]] + pattern)


def build_bass(cfg: Cfg, use_bias: bool):
    C, TROW, KC, ROWT = cfg.c, cfg.trow, cfg.kc, cfg.rowt
    WTC = 64 + C  # Wo head starts at partition 64 (PE base-partition rule)
    nc = bacc.Bacc("TRN2", target_bir_lowering=False, debug=False,
                   num_devices=cfg.cores)

    x_h = nc.dram_tensor("x", [cfg.s + cfg.u, cfg.d], BF16, kind="ExternalInput")
    wt_h = nc.dram_tensor("wt", [cfg.d, WTC], BF16, kind="ExternalInput")
    consts_h = nc.dram_tensor("consts", [128, 3 * C], F32, kind="ExternalInput")
    labelf_h = nc.dram_tensor("labelf", [128, cfg.lab_chunks], F32,
                              kind="ExternalInput")
    gidx_h = nc.dram_tensor("gidx", [128, cfg.chunks * 40], I16,
                            kind="ExternalInput")
    biascol_h = nc.dram_tensor("biascol", [WTC, 2], F32, kind="ExternalInput")
    out_h = nc.dram_tensor("out", [1, 1], F32, kind="ExternalOutput")

    rg = [list(range(cfg.cores))]
    W5 = cfg.chunks * 5

    with tile.TileContext(nc) as tc:
        ppcm = tc.tile_pool(name="persist", bufs=1)
        pp_ = ppcm.__enter__()

        def P(shape, dtype, name):
            return pp_.tile(shape, dtype, name=name, tag=name)

        # ---- persistent/constant SBUF (loads issued from Sync, first) ----
        wt_sb = P([128, KC, WTC], BF16, "wt_sb")
        nc.sync.dma_start(
            out=wt_sb[:],
            in_=bass.AP(tensor=wt_h, offset=0,
                        ap=[[WTC, 128], [128 * WTC, KC], [1, WTC]]))
        consts_sb = P([128, 3 * C], F32, "consts_sb")
        nc.sync.dma_start(out=consts_sb[:], in_=consts_h[:])
        iota_r = consts_sb[:, 0:C]
        gm_r = consts_sb[:, C:2 * C]
        gt_r = consts_sb[:, 2 * C:3 * C]
        labelf_sb = P([128, cfg.lab_chunks], F32, "labelf_sb")
        nc.sync.dma_start(out=labelf_sb[:], in_=labelf_h[:])
        gidx_sb = P([128, cfg.chunks * 40], I16, "gidx_sb")
        nc.sync.dma_start(out=gidx_sb[:], in_=gidx_h[:])
        ident = P([128, 128], F32, "ident")
        make_identity(nc, ident[:])
        ones128 = P([128, 1], F32, "ones128")
        nc.vector.memset(ones128[:], 1.0)
        lnwarm = P([1, 1], F32, "lnwarm")
        nc.scalar.activation(lnwarm[:], ones128[0:1, :], AF.Ln)
        if use_bias:
            biascol_sb = P([WTC, 2], F32, "biascol_sb")
            nc.sync.dma_start(out=biascol_sb[:], in_=biascol_h[:])

        zu_all = P([128, cfg.chunks, C], F32, "zu_all")
        ohu_all = P([128, cfg.chunks, C], F32, "ohu_all")
        wbuf = P([128, 2, cfg.chunks], F32, "wbuf")
        d1buf = P([128, W5], F32, "d1buf")
        dotbuf = P([128, W5], F32, "dotbuf")
        nmbuf = P([128, W5], F32, "nmbuf")   # -max(l) per pair (stable lse)

        with tc.tile_pool(name="dramp", bufs=1, space="DRAM") as dramp:
            t_local = dramp.tile([cfg.s, TROW], F32, name="t_local")
            t_full = dramp.tile([cfg.n_o, TROW], F32, name="t_full",
                                addr_space="Shared")
            p_local = dramp.tile([1, 2], F32, name="p_local")
            p_full = dramp.tile([cfg.cores, 2], F32, name="p_full",
                                addr_space="Shared")

            with (
                tc.tile_pool(name="xt", bufs=cfg.lab_tiles + cfg.unl_tiles)
                    as xt_pool,
                tc.tile_pool(name="ztp", bufs=2, space="PSUM") as zt_pool,
                tc.tile_pool(name="zts", bufs=2) as zts_pool,
                tc.tile_pool(name="trp", bufs=4, space="PSUM") as tr_pool,
                tc.tile_pool(name="ppp", bufs=1, space="PSUM") as pp_pool,
                tc.tile_pool(name="lrow", bufs=3) as lrow_pool,
                tc.tile_pool(name="small", bufs=8) as small_pool,
                tc.tile_pool(name="stat", bufs=16) as stat_pool,
                tc.tile_pool(name="gp", bufs=cfg.chunks) as g_pool,
                tc.tile_pool(name="wide", bufs=2) as wide_pool,
            ):
                # ---- all transposed feat loads, issued up front ----
                # labeled tiles first (they gate the AllGather), split
                # across both HWDGE engines
                nt = cfg.lab_tiles + cfg.unl_tiles
                xts = [None] * nt
                half = cfg.lab_tiles // 2
                sync_order = (list(range(half)) +
                              list(range(cfg.lab_tiles,
                                         cfg.lab_tiles + cfg.unl_tiles // 2)))
                scal_order = (list(range(half, cfg.lab_tiles)) +
                              list(range(cfg.lab_tiles + cfg.unl_tiles // 2, nt)))
                for eng, olist in ((nc.sync, sync_order), (nc.scalar, scal_order)):
                    for t in olist:
                        xt = xt_pool.tile([128, KC, ROWT], BF16, name="xt",
                                          tag="xt")
                        r0 = t * ROWT
                        eng.dma_start_transpose(xt[:], x_h[r0:r0 + ROWT, :])
                        xts[t] = xt

                def matmul_tile(xt, m, copy_eng):
                    zt = zt_pool.tile([m, ROWT], F32, tag="zt", name="zt")
                    for k in range(KC):
                        nc.tensor.matmul(
                            zt[:], lhsT=wt_sb[:, k, 0:m],
                            rhs=xt[:, k, :], start=(k == 0), stop=(k == KC - 1))
                    zts = zts_pool.tile([m, ROWT], F32, tag="zts", name="zts")
                    if use_bias:
                        col = 0 if m == C else 1
                        if copy_eng is nc.scalar:
                            nc.scalar.add(zts[:], zt[:],
                                          biascol_sb[0:m, col:col + 1])
                        else:
                            nc.vector.tensor_scalar(
                                out=zts[:], in0=zt[:],
                                scalar1=biascol_sb[0:m, col:col + 1],
                                scalar2=None, op0=ALU.add)
                    elif copy_eng is nc.scalar:
                        nc.scalar.copy(zts[:], zt[:])
                    else:
                        nc.vector.tensor_copy(zts[:], zt[:])
                    return zts

                # ================= Phase A: labeled =================
                for t in range(cfg.lab_tiles):
                    zts = matmul_tile(xts[t], C, nc.vector)
                    for q in range(cfg.cpt):
                        g = t * cfg.cpt + q
                        tr = tr_pool.tile([128, C], F32, tag="tr", name="tr")
                        nc.tensor.transpose(tr[:], zts[0:C, q * 128:(q + 1) * 128],
                                            ident[0:C, 0:C])
                        lt = lrow_pool.tile([128, 2 * C], F32, tag="lt",
                                            name="lt")
                        nc.vector.tensor_scalar_mul(lt[:, 0:C], tr[:], 0.7)
                        nc.vector.tensor_scalar(
                            out=lt[:, C:2 * C], in0=iota_r,
                            scalar1=labelf_sb[:, g:g + 1], scalar2=0.7,
                            op0=ALU.is_equal, op1=ALU.mult)
                        nc.sync.dma_start(
                            out=t_local[g * 128:(g + 1) * 128, 0:2 * C],
                            in_=lt[:])

                nc.gpsimd.collective_compute(
                    "AllGather", ALU.bypass, replica_groups=rg,
                    ins=[t_local[:].opt()], outs=[t_full[:].opt()])

                # ============ Phase 3a: issue all gathers early ============
                # 2 chunks (1280 rows) per dma_gather call to amortize the
                # SWDGE fixed cost; desc-gen rate is the phase-3 floor.
                call_groups = [[g] for g in range(cfg.chunks)]
                g_tiles = {}
                for grp in call_groups:
                    n_idx = 640 * len(grp)
                    gt_t = g_pool.tile([128, 5 * len(grp), TROW], F32,
                                       tag="g", name="gt_t")
                    c0 = grp[0] * 40
                    nc.gpsimd.dma_gather(
                        out_ap=gt_t[:], in_ap=t_full[:],
                        idxs_ap=gidx_sb[:, c0:c0 + n_idx // 16],
                        num_idxs=n_idx, num_idxs_reg=n_idx, elem_size=TROW)
                    for k, g in enumerate(grp):
                        g_tiles[g] = (gt_t, k)

                # ================= Phase B: unlabeled =================
                for t in range(cfg.unl_tiles):
                    zts = matmul_tile(xts[cfg.lab_tiles + t], WTC, nc.scalar)
                    for q in range(cfg.cpt):
                        g = t * cfg.cpt + q
                        trw = tr_pool.tile([128, C], F32, tag="tr", name="trw")
                        nc.tensor.transpose(trw[:], zts[0:C, q * 128:(q + 1) * 128],
                                            ident[0:C, 0:C])
                        tro = tr_pool.tile([128, C], F32, tag="tr", name="tro")
                        nc.tensor.transpose(tro[:],
                                            zts[64:64 + C, q * 128:(q + 1) * 128],
                                            ident[64:64 + C, 64:64 + C])
                        nc.vector.tensor_scalar_mul(zu_all[:, g, :], trw[:], 0.3)
                        negm = stat_pool.tile([128, 1], F32, tag="st", name="negm")
                        nc.vector.tensor_reduce(negm[:], tro[:], axis=AX.X,
                                                op=ALU.max, negate=True)
                        ej = small_pool.tile([128, C], F32, tag="sm", name="ej")
                        svec = stat_pool.tile([128, 1], F32, tag="st", name="svec")
                        nc.scalar.activation(ej[:], tro[:], AF.Exp,
                                             bias=negm[:], scale=1.0,
                                             accum_out=svec[:])
                        # onehot(pred) = ((lo + negm) == 0)
                        nc.vector.tensor_scalar(
                            out=ohu_all[:, g, :], in0=tro[:], scalar1=negm[:],
                            scalar2=0.0, op0=ALU.add, op1=ALU.is_equal)
                        gvm = stat_pool.tile([128, 1], F32, tag="st", name="gvm")
                        jm = small_pool.tile([128, C], F32, tag="sm", name="jm")
                        nc.vector.scalar_tensor_tensor(
                            out=jm[:], in0=ohu_all[:, g, :], scalar=1.0,
                            in1=gm_r, op0=ALU.mult, op1=ALU.mult,
                            accum_out=gvm[:])
                        gvt = stat_pool.tile([128, 1], F32, tag="st", name="gvt")
                        jt = small_pool.tile([128, C], F32, tag="sm", name="jt")
                        nc.vector.scalar_tensor_tensor(
                            out=jt[:], in0=ohu_all[:, g, :], scalar=1.0,
                            in1=gt_r, op0=ALU.mult, op1=ALU.mult,
                            accum_out=gvt[:])
                        nc.vector.scalar_tensor_tensor(
                            out=wbuf[:, 0, g:g + 1], in0=svec[:], scalar=2.0,
                            in1=gvm[:], op0=ALU.is_lt, op1=ALU.mult)
                        nc.vector.scalar_tensor_tensor(
                            out=wbuf[:, 1, g:g + 1], in0=svec[:],
                            scalar=float(1.0 / 0.3), in1=gvt[:],
                            op0=ALU.is_lt, op1=ALU.mult)

                # ================= Phase 3b: pair CE =================
                for g in range(cfg.chunks):
                    gt_full, koff = g_tiles[g]
                    gt_t = gt_full[:, koff * 5:(koff + 1) * 5, :]
                    g5 = g * 5
                    zub = _bc(zu_all[:], zu_all[:, g, :], [[0, 5], [1, C]])
                    ohb = _bc(ohu_all[:], ohu_all[:, g, :], [[0, 5], [1, C]])
                    lp = wide_pool.tile([128, 5, C], F32, tag="lp", name="lp")
                    nc.vector.tensor_tensor(out=lp[:], in0=gt_t[:, :, 0:C],
                                            in1=zub, op=ALU.add)
                    nc.vector.tensor_reduce(nmbuf[:, g5:g5 + 5], lp[:],
                                            axis=AX.X, op=ALU.max, negate=True)
                    lps = wide_pool.tile([128, 5, C], F32, tag="lps", name="lps")
                    nc.vector.tensor_tensor(
                        out=lps[:], in0=lp[:],
                        in1=_bc(nmbuf[:], nmbuf[:, g5:g5 + 5], [[1, 5], [0, C]]),
                        op=ALU.add)
                    ew = wide_pool.tile([128, 5, C], F32, tag="ew", name="ew")
                    nc.scalar.activation(ew[:], lps[:], AF.Exp)
                    nc.vector.tensor_reduce(d1buf[:, g5:g5 + 5], ew[:],
                                            axis=AX.X, op=ALU.add)
                    yw = wide_pool.tile([128, 5, C], F32, tag="yw", name="yw")
                    nc.vector.scalar_tensor_tensor(
                        out=yw[:], in0=ohb, scalar=0.3, in1=gt_t[:, :, C:2 * C],
                        op0=ALU.mult, op1=ALU.add)
                    pw = wide_pool.tile([128, 5, C], F32, tag="pw", name="pw")
                    nc.vector.tensor_tensor(out=pw[:], in0=lp[:], in1=yw[:],
                                            op=ALU.mult)
                    nc.vector.tensor_reduce(dotbuf[:, g5:g5 + 5], pw[:],
                                            axis=AX.X, op=ALU.add)

                # ================= Final reduction =================
                lse = P([128, W5], F32, "lse")
                nc.scalar.activation(lse[:], d1buf[:], AF.Ln)
                ce = P([128, W5], F32, "ce")
                nc.vector.tensor_tensor(out=ce[:], in0=lse[:], in1=nmbuf[:],
                                        op=ALU.subtract)   # lse + m
                nc.vector.tensor_tensor(out=ce[:], in0=ce[:], in1=dotbuf[:],
                                        op=ALU.subtract)
                accw = P([128, 2], F32, "accw")
                amid = P([128, 1], F32, "amid")
                jA = P([128, cfg.chunks, 2], F32, "jA")
                ce3 = bass.AP(tensor=ce[:].tensor, offset=ce[:].offset,
                              ap=[ce[:].ap[0], [5, cfg.chunks], [1, 2]])
                wA = _bc(wbuf[:], wbuf[:, 0, :], [[1, cfg.chunks], [0, 2]])
                nc.vector.scalar_tensor_tensor(
                    out=jA[:], in0=ce3, scalar=1.0, in1=wA,
                    op0=ALU.mult, op1=ALU.mult, accum_out=amid[:])
                atail = P([128, 1], F32, "atail")
                jB = P([128, cfg.chunks, 3], F32, "jB")
                ce2 = bass.AP(tensor=ce[:].tensor, offset=ce[:, 2:3].offset,
                              ap=[ce[:].ap[0], [5, cfg.chunks], [1, 3]])
                wB = _bc(wbuf[:], wbuf[:, 1, :], [[1, cfg.chunks], [0, 3]])
                nc.vector.scalar_tensor_tensor(
                    out=jB[:], in0=ce2, scalar=1.0, in1=wB,
                    op0=ALU.mult, op1=ALU.mult, accum_out=atail[:])
                nc.vector.tensor_tensor(out=accw[:, 0:1], in0=amid[:],
                                        in1=atail[:], op=ALU.add)
                # w_sum = 2*sum(midw) + 3*sum(tailw)
                smid = P([128, 1], F32, "smid")
                nc.vector.tensor_reduce(smid[:], wbuf[:, 0, :], axis=AX.X,
                                        op=ALU.add)
                stail = P([128, 1], F32, "stail")
                nc.vector.tensor_reduce(stail[:], wbuf[:, 1, :], axis=AX.X,
                                        op=ALU.add)
                st3 = P([128, 1], F32, "st3")
                nc.vector.tensor_scalar_mul(st3[:], stail[:], 3.0)
                nc.vector.scalar_tensor_tensor(
                    out=accw[:, 1:2], in0=smid[:], scalar=2.0, in1=st3[:],
                    op0=ALU.mult, op1=ALU.add)
                pp = pp_pool.tile([1, 2], F32, name="pp")
                nc.tensor.matmul(pp[:], lhsT=ones128[:], rhs=accw[:],
                                 start=True, stop=True)
                ppsb = P([1, 2], F32, "ppsb")
                nc.vector.tensor_copy(ppsb[:], pp[:])
                nc.sync.dma_start(out=p_local[:], in_=ppsb[:])
                nc.gpsimd.collective_compute(
                    "AllGather", ALU.bypass, replica_groups=rg,
                    ins=[p_local[:].opt()], outs=[p_full[:].opt()])
                pf = P([1, 2 * cfg.cores], F32, "pf")
                nc.sync.dma_start(
                    out=pf[:],
                    in_=bass.AP(tensor=p_full[:].tensor, offset=p_full[:].offset,
                                ap=[[0, 1], [1, 2 * cfg.cores]]))
                red = P([1, 2], F32, "red")
                nc.vector.tensor_reduce(
                    red[:],
                    bass.AP(tensor=pf[:].tensor, offset=pf[:].offset,
                            ap=[pf[:].ap[0], [1, 2], [2, cfg.cores]]),
                    axis=AX.X, op=ALU.add)
                cmax = P([1, 1], F32, "cmax")
                nc.vector.tensor_scalar_max(cmax[:], red[:, 1:2], 1.0)
                rec = P([1, 1], F32, "rec")
                nc.vector.reciprocal(rec[:], cmax[:])
                fin = P([1, 1], F32, "fin")
                nc.vector.tensor_tensor(out=fin[:], in0=red[:, 0:1], in1=rec[:],
                                        op=ALU.mult)
                nc.sync.dma_start(out=out_h[:], in_=fin[:])

        ppcm.__exit__(None, None, None)

    nc.compile()
    return nc


def make_in_maps(cfg: Cfg, feat, label, W_o, b_o, W, b, gm, gt, idx_m, idx_t):
    """Host-side shard/prep. Returns (in_maps, use_bias)."""
    n_o, C = cfg.n_o, cfg.c
    feat = np.ascontiguousarray(np.asarray(feat, np.float32))
    label = np.asarray(label).astype(np.int64)
    W_o = np.asarray(W_o, np.float32)
    W = np.asarray(W, np.float32)
    b_o = np.asarray(b_o, np.float32)
    b = np.asarray(b, np.float32)
    gm = np.asarray(gm).astype(np.float32)
    gt = np.asarray(gt).astype(np.float32)
    idxs = np.concatenate([np.asarray(idx_m), np.asarray(idx_t)], 0).astype(np.int64)

    use_bias = bool(np.any(b) or np.any(b_o))
    feat_bf = feat.astype(ml_dtypes.bfloat16)
    wt = np.zeros((cfg.d, 64 + C), np.float32)
    wt[:, 0:C] = W.T
    wt[:, 64:64 + C] = W_o.T
    wt = np.ascontiguousarray(wt.astype(ml_dtypes.bfloat16))
    consts = np.concatenate([
        np.tile(np.arange(C, dtype=np.float32), (128, 1)),
        np.tile(gm, (128, 1)),
        np.tile(gt, (128, 1)),
    ], axis=1)
    consts = np.ascontiguousarray(consts)
    biascol = np.zeros((64 + C, 2), np.float32)
    biascol[0:C, 0] = b / 0.7
    biascol[64:64 + C, 1] = b_o
    label_o = label[:n_o].astype(np.float32)

    in_maps = []
    for r in range(cfg.cores):
        lab0, unl0 = cfg.s * r, n_o + cfg.u * r
        x = np.concatenate([feat_bf[lab0:lab0 + cfg.s],
                            feat_bf[unl0:unl0 + cfg.u]], axis=0)
        labelf = label_o[lab0:lab0 + cfg.s].reshape(cfg.lab_chunks, 128).T
        gcols = []
        for a in range(0, cfg.chunks, 1):
            grp = [a]
            flats = []
            for g in grp:
                rows = cfg.u * r + g * 128 + np.arange(128)
                flats.append(idxs[:, rows].reshape(-1))   # [5*128] c-major
            flat = np.concatenate(flats)                  # [640*len(grp)]
            a16 = flat.reshape(-1, 16).T                  # [16, 40*len]
            gcols.append(np.tile(a16, (8, 1)))
        gidx = np.concatenate(gcols, axis=1).astype(np.int16)
        in_maps.append(dict(
            x=np.ascontiguousarray(x),
            wt=wt,
            consts=consts,
            labelf=np.ascontiguousarray(labelf.astype(np.float32)),
            gidx=np.ascontiguousarray(gidx),
            biascol=biascol,
        ))
    return in_maps, use_bias


_CACHE = {}


def _get_nc(cfg: Cfg, use_bias: bool):
    key = (cfg.n_o, cfg.n_u, cfg.d, cfg.cores, cfg.rowt, use_bias)
    if key not in _CACHE:
        _CACHE[key] = build_bass(cfg, use_bias)
    return _CACHE[key]


def _install_ntff_shim():
    """This image's antenv lacks axon_hooks; recreate it so trace=True works."""
    import sys
    import types
    try:
        from antenv.axon_hooks import get_axon_ntff_profile_hook  # noqa: F401
        return
    except ImportError:
        pass
    try:
        import antenv
        from trn_agent_boot.trn_boot import _ntff_profile_via_ctypes
        h = _ntff_profile_via_ctypes("/opt/axon/libaxon_pjrt.so")
        mod = types.ModuleType("antenv.axon_hooks")
        mod.get_axon_ntff_profile_hook = lambda: h
        mod.set_axon_ntff_profile_hook = lambda hook: None
        sys.modules["antenv.axon_hooks"] = mod
        antenv.axon_hooks = mod
    except Exception:
        pass


def kernel(feat, label, W_o, b_o, W, b, group_mid_mask, group_tail_mask,
           idx_m, idx_t, _trace=False):
    if _trace:
        _install_ntff_shim()
    n_u = int(np.asarray(idx_m).shape[1])
    n_o = int(np.asarray(feat).shape[0]) - n_u
    cfg = Cfg(n_o=n_o, n_u=n_u, d=int(np.asarray(feat).shape[1]))
    in_maps, use_bias = make_in_maps(cfg, feat, label, W_o, b_o, W, b,
                                     group_mid_mask, group_tail_mask,
                                     idx_m, idx_t)
    nc = _get_nc(cfg, use_bias)
    res = run_bass_kernel_spmd(nc, in_maps, core_ids=list(range(cfg.cores)),
                               trace=_trace)
    out = np.float32(res.results[0]["out"].reshape(-1)[0])
    if _trace:
        return out, res
    return out


# revision 17
# speedup vs baseline: 1.0695x; 1.0695x over previous
"""Trainium2 Bass kernel for nn_BalanceLabelAugmentation2 (topk_masking).

Math (reference, restructured):
  Z   = feat @ W.T            [N, 51]   (matmul is linear over the mixup!)
  lo  = feat_u @ W_o.T + b_o  [N_u, 51] -> pred=argmax, score=max softmax
  midw_i  = gm[pred_i] & (score_i > 0.5);  tailw_i = gt[pred_i] & (score_i > 0.3)
  For pair (copy c, unlabeled row i) with partner j = idx_c[i]:
    l    = 0.7*Z_o[j] + b + 0.3*Z_u[i]
    ce   = logsumexp(l) - sum(l * (0.7*onehot(label_j) + 0.3*onehot(pred_i)))
  out = sum(ce*w) / max(sum w, 1)

Distribution (8 cores, data-parallel rows):
  core r owns labeled rows [2048r, 2048(r+1)) and unlabeled rows likewise.
  Phase A: matmul labeled shard -> table row j = [L_j=0.7*Z_o[j]+b |
           0.7*onehot(label_j) | pad] (f32, 512B rows), AllGather the table.
  Phase B: matmul unlabeled shard (both heads) -> ZU=0.3*Z_u, onehot(pred),
           score, masks.
  Phase 3: dma_gather table rows for the core's 5*2048 pairs (640 rows/chunk,
           issued back-to-back on GpSimd right after the AllGather), fused
           DVE/ACT soft-CE with stable logsumexp, weighted accumulate.
  Final:   per-core [ce_sum, w_sum] -> AllGather -> each core computes scalar.

feat is cast to bf16 on the host (halves DMA, enables the HW xbar
DMA-transpose loads; ~1e-5 end-to-end effect on the reference inputs).
All post-matmul math is f32.

Engine placement: GpSimd = collectives + the 16 dma_gathers (desc-gen is the
phase-3 floor) + constant loads; labeled-shard transposed loads on Sync,
unlabeled on Scalar, all emitted up front so they stream ahead of compute.
"""

import numpy as np
import ml_dtypes

import concourse.bass as bass
import concourse.tile as tile
from concourse import bacc, mybir
from concourse.bass_utils import run_bass_kernel_spmd
from concourse.masks import make_identity

F32 = mybir.dt.float32
BF16 = mybir.dt.bfloat16
I16 = mybir.dt.int16
AF = mybir.ActivationFunctionType
ALU = mybir.AluOpType
AX = mybir.AxisListType


class Cfg:
    def __init__(self, n_o=16384, n_u=16384, d=1024, cores=8, rowt=512):
        self.n_o, self.n_u, self.d, self.cores, self.rowt = n_o, n_u, d, cores, rowt
        self.c = 51
        self.s = n_o // cores          # labeled rows per core
        self.u = n_u // cores          # unlabeled rows per core
        self.kc = d // 128             # contraction chunks
        self.lab_tiles = self.s // rowt
        self.unl_tiles = self.u // rowt
        self.cpt = rowt // 128         # 128-row chunks per tile
        self.lab_chunks = self.s // 128
        self.chunks = self.u // 128    # unlabeled 128-row chunks
        self.trow = 128                # table row f32 elems (512B; %256B for gather)
        assert self.s % rowt == 0 and self.u % rowt == 0 and d % 128 == 0


def _bc(tile_ap, offset_ap, pattern):
    """AP on tile_ap's tensor at offset_ap's offset with a custom free pattern."""
    return bass.AP(tensor=tile_ap.tensor, offset=offset_ap.offset,
                   ap=[tile_ap.ap[0]] + pattern)


def build_bass(cfg: Cfg, use_bias: bool):
    C, TROW, KC, ROWT = cfg.c, cfg.trow, cfg.kc, cfg.rowt
    WTC = 64 + C  # Wo head starts at partition 64 (PE base-partition rule)
    nc = bacc.Bacc("TRN2", target_bir_lowering=False, debug=False,
                   num_devices=cfg.cores)

    x_h = nc.dram_tensor("x", [cfg.s + cfg.u, cfg.d], BF16, kind="ExternalInput")
    wt_h = nc.dram_tensor("wt", [cfg.d, WTC], BF16, kind="ExternalInput")
    consts_h = nc.dram_tensor("consts", [128, 3 * C], F32, kind="ExternalInput")
    labelf_h = nc.dram_tensor("labelf", [128, cfg.lab_chunks], F32,
                              kind="ExternalInput")
    gidx_h = nc.dram_tensor("gidx", [128, cfg.chunks * 40], I16,
                            kind="ExternalInput")
    biascol_h = nc.dram_tensor("biascol", [WTC, 2], F32, kind="ExternalInput")
    out_h = nc.dram_tensor("out", [1, 1], F32, kind="ExternalOutput")

    rg = [list(range(cfg.cores))]
    W5 = cfg.chunks * 5

    with tile.TileContext(nc) as tc:
        ppcm = tc.tile_pool(name="persist", bufs=1)
        pp_ = ppcm.__enter__()

        def P(shape, dtype, name):
            return pp_.tile(shape, dtype, name=name, tag=name)

        # ---- persistent/constant SBUF (loads issued from Sync, first) ----
        wt_sb = P([128, KC, WTC], BF16, "wt_sb")
        nc.sync.dma_start(
            out=wt_sb[:],
            in_=bass.AP(tensor=wt_h, offset=0,
                        ap=[[WTC, 128], [128 * WTC, KC], [1, WTC]]))
        consts_sb = P([128, 3 * C], F32, "consts_sb")
        nc.sync.dma_start(out=consts_sb[:], in_=consts_h[:])
        iota_r = consts_sb[:, 0:C]
        gm_r = consts_sb[:, C:2 * C]
        gt_r = consts_sb[:, 2 * C:3 * C]
        labelf_sb = P([128, cfg.lab_chunks], F32, "labelf_sb")
        nc.sync.dma_start(out=labelf_sb[:], in_=labelf_h[:])
        gidx_sb = P([128, cfg.chunks * 40], I16, "gidx_sb")
        nc.sync.dma_start(out=gidx_sb[:], in_=gidx_h[:])
        ident = P([128, 128], F32, "ident")
        make_identity(nc, ident[:])
        ones128 = P([128, 1], F32, "ones128")
        nc.vector.memset(ones128[:], 1.0)
        if use_bias:
            biascol_sb = P([WTC, 2], F32, "biascol_sb")
            nc.sync.dma_start(out=biascol_sb[:], in_=biascol_h[:])

        zu_all = P([128, cfg.chunks, C], F32, "zu_all")
        ohu_all = P([128, cfg.chunks, C], F32, "ohu_all")
        wbuf = P([128, 2, cfg.chunks], F32, "wbuf")
        d1buf = P([128, W5], F32, "d1buf")
        dotbuf = P([128, W5], F32, "dotbuf")
        nmbuf = P([128, W5], F32, "nmbuf")   # -max(l) per pair (stable lse)

        with tc.tile_pool(name="dramp", bufs=1, space="DRAM") as dramp:
            t_local = dramp.tile([cfg.s, TROW], BF16, name="t_local")
            t_full = dramp.tile([cfg.n_o, TROW], BF16, name="t_full",
                                addr_space="Shared")
            p_local = dramp.tile([1, 2], F32, name="p_local")
            p_full = dramp.tile([cfg.cores, 2], F32, name="p_full",
                                addr_space="Shared")

            with (
                tc.tile_pool(name="xt", bufs=cfg.lab_tiles + cfg.unl_tiles)
                    as xt_pool,
                tc.tile_pool(name="ztp", bufs=2, space="PSUM") as zt_pool,
                tc.tile_pool(name="zts", bufs=2) as zts_pool,
                tc.tile_pool(name="trp", bufs=4, space="PSUM") as tr_pool,
                tc.tile_pool(name="ppp", bufs=1, space="PSUM") as pp_pool,
                tc.tile_pool(name="lrow", bufs=3) as lrow_pool,
                tc.tile_pool(name="small", bufs=8) as small_pool,
                tc.tile_pool(name="stat", bufs=16) as stat_pool,
                tc.tile_pool(name="gp", bufs=cfg.chunks) as g_pool,
                tc.tile_pool(name="wide", bufs=2) as wide_pool,
            ):
                # ---- all transposed feat loads, issued up front ----
                # labeled tiles first (they gate the AllGather), split
                # across both HWDGE engines
                nt = cfg.lab_tiles + cfg.unl_tiles
                xts = [None] * nt
                half = cfg.lab_tiles // 2
                sync_order = (list(range(half)) +
                              list(range(cfg.lab_tiles,
                                         cfg.lab_tiles + cfg.unl_tiles // 2)))
                scal_order = (list(range(half, cfg.lab_tiles)) +
                              list(range(cfg.lab_tiles + cfg.unl_tiles // 2, nt)))
                for eng, olist in ((nc.sync, sync_order), (nc.scalar, scal_order)):
                    for t in olist:
                        xt = xt_pool.tile([128, KC, ROWT], BF16, name="xt",
                                          tag="xt")
                        r0 = t * ROWT
                        eng.dma_start_transpose(xt[:], x_h[r0:r0 + ROWT, :])
                        xts[t] = xt

                def matmul_tile(xt, m, copy_eng):
                    zt = zt_pool.tile([m, ROWT], F32, tag="zt", name="zt")
                    for k in range(KC):
                        nc.tensor.matmul(
                            zt[:], lhsT=wt_sb[:, k, 0:m],
                            rhs=xt[:, k, :], start=(k == 0), stop=(k == KC - 1))
                    zts = zts_pool.tile([m, ROWT], F32, tag="zts", name="zts")
                    if use_bias:
                        col = 0 if m == C else 1
                        if copy_eng is nc.scalar:
                            nc.scalar.add(zts[:], zt[:],
                                          biascol_sb[0:m, col:col + 1])
                        else:
                            nc.vector.tensor_scalar(
                                out=zts[:], in0=zt[:],
                                scalar1=biascol_sb[0:m, col:col + 1],
                                scalar2=None, op0=ALU.add)
                    elif copy_eng is nc.scalar:
                        nc.scalar.copy(zts[:], zt[:])
                    else:
                        nc.vector.tensor_copy(zts[:], zt[:])
                    return zts

                # ================= Phase A: labeled =================
                for t in range(cfg.lab_tiles):
                    zts = matmul_tile(xts[t], C, nc.vector)
                    for q in range(cfg.cpt):
                        g = t * cfg.cpt + q
                        tr = tr_pool.tile([128, C], F32, tag="tr", name="tr")
                        nc.tensor.transpose(tr[:], zts[0:C, q * 128:(q + 1) * 128],
                                            ident[0:C, 0:C])
                        lt = lrow_pool.tile([128, 2 * C], BF16, tag="lt",
                                            name="lt")
                        nc.vector.tensor_scalar_mul(lt[:, 0:C], tr[:], 0.7)
                        nc.vector.tensor_scalar(
                            out=lt[:, C:2 * C], in0=iota_r,
                            scalar1=labelf_sb[:, g:g + 1], scalar2=None,
                            op0=ALU.is_equal)
                        nc.sync.dma_start(
                            out=t_local[g * 128:(g + 1) * 128, 0:2 * C],
                            in_=lt[:])

                nc.gpsimd.collective_compute(
                    "AllGather", ALU.bypass, replica_groups=rg,
                    ins=[t_local[:].opt()], outs=[t_full[:].opt()])

                # ============ Phase 3a: issue all gathers early ============
                # 2 chunks (1280 rows) per dma_gather call to amortize the
                # SWDGE fixed cost; desc-gen rate is the phase-3 floor.
                call_groups = [[g] for g in range(cfg.chunks)]
                g_tiles = {}
                for grp in call_groups:
                    n_idx = 640 * len(grp)
                    gt_t = g_pool.tile([128, 5 * len(grp), TROW], BF16,
                                       tag="g", name="gt_t")
                    c0 = grp[0] * 40
                    nc.gpsimd.dma_gather(
                        out_ap=gt_t[:], in_ap=t_full[:],
                        idxs_ap=gidx_sb[:, c0:c0 + n_idx // 16],
                        num_idxs=n_idx, num_idxs_reg=n_idx, elem_size=TROW)
                    for k, g in enumerate(grp):
                        g_tiles[g] = (gt_t, k)

                # ================= Phase B: unlabeled =================
                for t in range(cfg.unl_tiles):
                    zts = matmul_tile(xts[cfg.lab_tiles + t], WTC, nc.scalar)
                    for q in range(cfg.cpt):
                        g = t * cfg.cpt + q
                        trw = tr_pool.tile([128, C], F32, tag="tr", name="trw")
                        nc.tensor.transpose(trw[:], zts[0:C, q * 128:(q + 1) * 128],
                                            ident[0:C, 0:C])
                        tro = tr_pool.tile([128, C], F32, tag="tr", name="tro")
                        nc.tensor.transpose(tro[:],
                                            zts[64:64 + C, q * 128:(q + 1) * 128],
                                            ident[64:64 + C, 64:64 + C])
                        nc.vector.tensor_scalar_mul(zu_all[:, g, :], trw[:], 0.3)
                        negm = stat_pool.tile([128, 1], F32, tag="st", name="negm")
                        nc.vector.tensor_reduce(negm[:], tro[:], axis=AX.X,
                                                op=ALU.max, negate=True)
                        ej = small_pool.tile([128, C], F32, tag="sm", name="ej")
                        svec = stat_pool.tile([128, 1], F32, tag="st", name="svec")
                        nc.scalar.activation(ej[:], tro[:], AF.Exp,
                                             bias=negm[:], scale=1.0,
                                             accum_out=svec[:])
                        # onehot(pred) = ((lo + negm) == 0), then scale 0.3
                        oh0 = small_pool.tile([128, C], F32, tag="sm", name="oh0")
                        nc.vector.tensor_scalar(
                            out=oh0[:], in0=tro[:], scalar1=negm[:],
                            scalar2=0.0, op0=ALU.add, op1=ALU.is_equal)
                        nc.vector.tensor_scalar_mul(ohu_all[:, g, :], oh0[:], 0.3)
                        gvm = stat_pool.tile([128, 1], F32, tag="st", name="gvm")
                        jm = small_pool.tile([128, C], F32, tag="sm", name="jm")
                        nc.vector.scalar_tensor_tensor(
                            out=jm[:], in0=oh0[:], scalar=1.0,
                            in1=gm_r, op0=ALU.mult, op1=ALU.mult,
                            accum_out=gvm[:])
                        gvt = stat_pool.tile([128, 1], F32, tag="st", name="gvt")
                        jt = small_pool.tile([128, C], F32, tag="sm", name="jt")
                        nc.vector.scalar_tensor_tensor(
                            out=jt[:], in0=oh0[:], scalar=1.0,
                            in1=gt_r, op0=ALU.mult, op1=ALU.mult,
                            accum_out=gvt[:])
                        nc.vector.scalar_tensor_tensor(
                            out=wbuf[:, 0, g:g + 1], in0=svec[:], scalar=2.0,
                            in1=gvm[:], op0=ALU.is_lt, op1=ALU.mult)
                        nc.vector.scalar_tensor_tensor(
                            out=wbuf[:, 1, g:g + 1], in0=svec[:],
                            scalar=float(1.0 / 0.3), in1=gvt[:],
                            op0=ALU.is_lt, op1=ALU.mult)

                # ================= Phase 3b: pair CE =================
                for g in range(cfg.chunks):
                    gt_full, koff = g_tiles[g]
                    gt_t = gt_full[:, koff * 5:(koff + 1) * 5, :]
                    g5 = g * 5
                    zub = _bc(zu_all[:], zu_all[:, g, :], [[0, 5], [1, C]])
                    ohb = _bc(ohu_all[:], ohu_all[:, g, :], [[0, 5], [1, C]])
                    lp = wide_pool.tile([128, 5, C], F32, tag="lp", name="lp")
                    nc.vector.tensor_tensor(out=lp[:], in0=gt_t[:, :, 0:C],
                                            in1=zub, op=ALU.add)
                    nc.vector.tensor_reduce(nmbuf[:, g5:g5 + 5], lp[:],
                                            axis=AX.X, op=ALU.max, negate=True)
                    lps = wide_pool.tile([128, 5, C], F32, tag="lps", name="lps")
                    nc.vector.tensor_tensor(
                        out=lps[:], in0=lp[:],
                        in1=_bc(nmbuf[:], nmbuf[:, g5:g5 + 5], [[1, 5], [0, C]]),
                        op=ALU.add)
                    ew = wide_pool.tile([128, 5, C], F32, tag="ew", name="ew")
                    nc.scalar.activation(ew[:], lps[:], AF.Exp)
                    nc.vector.tensor_reduce(d1buf[:, g5:g5 + 5], ew[:],
                                            axis=AX.X, op=ALU.add)
                    yw = wide_pool.tile([128, 5, C], F32, tag="yw", name="yw")
                    nc.vector.scalar_tensor_tensor(
                        out=yw[:], in0=gt_t[:, :, C:2 * C], scalar=0.7,
                        in1=ohb, op0=ALU.mult, op1=ALU.add)
                    pw = wide_pool.tile([128, 5, C], F32, tag="pw", name="pw")
                    nc.vector.tensor_tensor(out=pw[:], in0=lp[:], in1=yw[:],
                                            op=ALU.mult)
                    nc.vector.tensor_reduce(dotbuf[:, g5:g5 + 5], pw[:],
                                            axis=AX.X, op=ALU.add)

                # ================= Final reduction =================
                lse = P([128, W5], F32, "lse")
                nc.scalar.activation(lse[:], d1buf[:], AF.Ln)
                ce = P([128, W5], F32, "ce")
                nc.vector.tensor_tensor(out=ce[:], in0=lse[:], in1=nmbuf[:],
                                        op=ALU.subtract)   # lse + m
                nc.vector.tensor_tensor(out=ce[:], in0=ce[:], in1=dotbuf[:],
                                        op=ALU.subtract)
                accw = P([128, 2], F32, "accw")
                amid = P([128, 1], F32, "amid")
                jA = P([128, cfg.chunks, 2], F32, "jA")
                ce3 = bass.AP(tensor=ce[:].tensor, offset=ce[:].offset,
                              ap=[ce[:].ap[0], [5, cfg.chunks], [1, 2]])
                wA = _bc(wbuf[:], wbuf[:, 0, :], [[1, cfg.chunks], [0, 2]])
                nc.vector.scalar_tensor_tensor(
                    out=jA[:], in0=ce3, scalar=1.0, in1=wA,
                    op0=ALU.mult, op1=ALU.mult, accum_out=amid[:])
                atail = P([128, 1], F32, "atail")
                jB = P([128, cfg.chunks, 3], F32, "jB")
                ce2 = bass.AP(tensor=ce[:].tensor, offset=ce[:, 2:3].offset,
                              ap=[ce[:].ap[0], [5, cfg.chunks], [1, 3]])
                wB = _bc(wbuf[:], wbuf[:, 1, :], [[1, cfg.chunks], [0, 3]])
                nc.vector.scalar_tensor_tensor(
                    out=jB[:], in0=ce2, scalar=1.0, in1=wB,
                    op0=ALU.mult, op1=ALU.mult, accum_out=atail[:])
                nc.vector.tensor_tensor(out=accw[:, 0:1], in0=amid[:],
                                        in1=atail[:], op=ALU.add)
                # w_sum = 2*sum(midw) + 3*sum(tailw)
                smid = P([128, 1], F32, "smid")
                nc.vector.tensor_reduce(smid[:], wbuf[:, 0, :], axis=AX.X,
                                        op=ALU.add)
                stail = P([128, 1], F32, "stail")
                nc.vector.tensor_reduce(stail[:], wbuf[:, 1, :], axis=AX.X,
                                        op=ALU.add)
                st3 = P([128, 1], F32, "st3")
                nc.vector.tensor_scalar_mul(st3[:], stail[:], 3.0)
                nc.vector.scalar_tensor_tensor(
                    out=accw[:, 1:2], in0=smid[:], scalar=2.0, in1=st3[:],
                    op0=ALU.mult, op1=ALU.add)
                pp = pp_pool.tile([1, 2], F32, name="pp")
                nc.tensor.matmul(pp[:], lhsT=ones128[:], rhs=accw[:],
                                 start=True, stop=True)
                ppsb = P([1, 2], F32, "ppsb")
                nc.vector.tensor_copy(ppsb[:], pp[:])
                nc.sync.dma_start(out=p_local[:], in_=ppsb[:])
                nc.gpsimd.collective_compute(
                    "AllGather", ALU.bypass, replica_groups=rg,
                    ins=[p_local[:].opt()], outs=[p_full[:].opt()])
                pf = P([1, 2 * cfg.cores], F32, "pf")
                nc.sync.dma_start(
                    out=pf[:],
                    in_=bass.AP(tensor=p_full[:].tensor, offset=p_full[:].offset,
                                ap=[[0, 1], [1, 2 * cfg.cores]]))
                red = P([1, 2], F32, "red")
                nc.vector.tensor_reduce(
                    red[:],
                    bass.AP(tensor=pf[:].tensor, offset=pf[:].offset,
                            ap=[pf[:].ap[0], [1, 2], [2, cfg.cores]]),
                    axis=AX.X, op=ALU.add)
                cmax = P([1, 1], F32, "cmax")
                nc.vector.tensor_scalar_max(cmax[:], red[:, 1:2], 1.0)
                rec = P([1, 1], F32, "rec")
                nc.vector.reciprocal(rec[:], cmax[:])
                fin = P([1, 1], F32, "fin")
                nc.vector.tensor_tensor(out=fin[:], in0=red[:, 0:1], in1=rec[:],
                                        op=ALU.mult)
                nc.sync.dma_start(out=out_h[:], in_=fin[:])

        ppcm.__exit__(None, None, None)

    nc.compile()
    return nc


def make_in_maps(cfg: Cfg, feat, label, W_o, b_o, W, b, gm, gt, idx_m, idx_t):
    """Host-side shard/prep. Returns (in_maps, use_bias)."""
    n_o, C = cfg.n_o, cfg.c
    feat = np.ascontiguousarray(np.asarray(feat, np.float32))
    label = np.asarray(label).astype(np.int64)
    W_o = np.asarray(W_o, np.float32)
    W = np.asarray(W, np.float32)
    b_o = np.asarray(b_o, np.float32)
    b = np.asarray(b, np.float32)
    gm = np.asarray(gm).astype(np.float32)
    gt = np.asarray(gt).astype(np.float32)
    idxs = np.concatenate([np.asarray(idx_m), np.asarray(idx_t)], 0).astype(np.int64)

    use_bias = bool(np.any(b) or np.any(b_o))
    feat_bf = feat.astype(ml_dtypes.bfloat16)
    wt = np.zeros((cfg.d, 64 + C), np.float32)
    wt[:, 0:C] = W.T
    wt[:, 64:64 + C] = W_o.T
    wt = np.ascontiguousarray(wt.astype(ml_dtypes.bfloat16))
    consts = np.concatenate([
        np.tile(np.arange(C, dtype=np.float32), (128, 1)),
        np.tile(gm, (128, 1)),
        np.tile(gt, (128, 1)),
    ], axis=1)
    consts = np.ascontiguousarray(consts)
    biascol = np.zeros((64 + C, 2), np.float32)
    biascol[0:C, 0] = b / 0.7
    biascol[64:64 + C, 1] = b_o
    label_o = label[:n_o].astype(np.float32)

    in_maps = []
    for r in range(cfg.cores):
        lab0, unl0 = cfg.s * r, n_o + cfg.u * r
        x = np.concatenate([feat_bf[lab0:lab0 + cfg.s],
                            feat_bf[unl0:unl0 + cfg.u]], axis=0)
        labelf = label_o[lab0:lab0 + cfg.s].reshape(cfg.lab_chunks, 128).T
        gcols = []
        for a in range(0, cfg.chunks, 1):
            grp = [a]
            flats = []
            for g in grp:
                rows = cfg.u * r + g * 128 + np.arange(128)
                flats.append(idxs[:, rows].reshape(-1))   # [5*128] c-major
            flat = np.concatenate(flats)                  # [640*len(grp)]
            a16 = flat.reshape(-1, 16).T                  # [16, 40*len]
            gcols.append(np.tile(a16, (8, 1)))
        gidx = np.concatenate(gcols, axis=1).astype(np.int16)
        in_maps.append(dict(
            x=np.ascontiguousarray(x),
            wt=wt,
            consts=consts,
            labelf=np.ascontiguousarray(labelf.astype(np.float32)),
            gidx=np.ascontiguousarray(gidx),
            biascol=biascol,
        ))
    return in_maps, use_bias


_CACHE = {}


def _get_nc(cfg: Cfg, use_bias: bool):
    key = (cfg.n_o, cfg.n_u, cfg.d, cfg.cores, cfg.rowt, use_bias)
    if key not in _CACHE:
        _CACHE[key] = build_bass(cfg, use_bias)
    return _CACHE[key]


def _install_ntff_shim():
    """This image's antenv lacks axon_hooks; recreate it so trace=True works."""
    import sys
    import types
    try:
        from antenv.axon_hooks import get_axon_ntff_profile_hook  # noqa: F401
        return
    except ImportError:
        pass
    try:
        import antenv
        from trn_agent_boot.trn_boot import _ntff_profile_via_ctypes
        h = _ntff_profile_via_ctypes("/opt/axon/libaxon_pjrt.so")
        mod = types.ModuleType("antenv.axon_hooks")
        mod.get_axon_ntff_profile_hook = lambda: h
        mod.set_axon_ntff_profile_hook = lambda hook: None
        sys.modules["antenv.axon_hooks"] = mod
        antenv.axon_hooks = mod
    except Exception:
        pass


def kernel(feat, label, W_o, b_o, W, b, group_mid_mask, group_tail_mask,
           idx_m, idx_t, _trace=False):
    if _trace:
        _install_ntff_shim()
    n_u = int(np.asarray(idx_m).shape[1])
    n_o = int(np.asarray(feat).shape[0]) - n_u
    cfg = Cfg(n_o=n_o, n_u=n_u, d=int(np.asarray(feat).shape[1]))
    in_maps, use_bias = make_in_maps(cfg, feat, label, W_o, b_o, W, b,
                                     group_mid_mask, group_tail_mask,
                                     idx_m, idx_t)
    nc = _get_nc(cfg, use_bias)
    res = run_bass_kernel_spmd(nc, in_maps, core_ids=list(range(cfg.cores)),
                               trace=_trace)
    out = np.float32(res.results[0]["out"].reshape(-1)[0])
    if _trace:
        return out, res
    return out


# revision 18
# speedup vs baseline: 1.0697x; 1.0001x over previous
"""Trainium2 Bass kernel for nn_BalanceLabelAugmentation2 (topk_masking).

Math (reference, restructured):
  Z   = feat @ W.T            [N, 51]   (matmul is linear over the mixup!)
  lo  = feat_u @ W_o.T + b_o  [N_u, 51] -> pred=argmax, score=max softmax
  midw_i  = gm[pred_i] & (score_i > 0.5);  tailw_i = gt[pred_i] & (score_i > 0.3)
  For pair (copy c, unlabeled row i) with partner j = idx_c[i]:
    l    = 0.7*Z_o[j] + b + 0.3*Z_u[i]
    ce   = logsumexp(l) - sum(l * (0.7*onehot(label_j) + 0.3*onehot(pred_i)))
  out = sum(ce*w) / max(sum w, 1)

Distribution (8 cores, data-parallel rows):
  core r owns labeled rows [2048r, 2048(r+1)) and unlabeled rows likewise.
  Phase A: matmul labeled shard -> table row j = [L_j=0.7*Z_o[j]+b |
           0.7*onehot(label_j) | pad] (f32, 512B rows), AllGather the table.
  Phase B: matmul unlabeled shard (both heads) -> ZU=0.3*Z_u, onehot(pred),
           score, masks.
  Phase 3: dma_gather table rows for the core's 5*2048 pairs (640 rows/chunk,
           issued back-to-back on GpSimd right after the AllGather), fused
           DVE/ACT soft-CE with stable logsumexp, weighted accumulate.
  Final:   per-core [ce_sum, w_sum] -> AllGather -> each core computes scalar.

feat is cast to bf16 on the host (halves DMA, enables the HW xbar
DMA-transpose loads; ~1e-5 end-to-end effect on the reference inputs).
All post-matmul math is f32.

Engine placement: GpSimd = collectives + the 16 dma_gathers (desc-gen is the
phase-3 floor) + constant loads; labeled-shard transposed loads on Sync,
unlabeled on Scalar, all emitted up front so they stream ahead of compute.
"""

import numpy as np
import ml_dtypes

import concourse.bass as bass
import concourse.tile as tile
from concourse import bacc, mybir
from concourse.bass_utils import run_bass_kernel_spmd
from concourse.masks import make_identity

F32 = mybir.dt.float32
BF16 = mybir.dt.bfloat16
I16 = mybir.dt.int16
AF = mybir.ActivationFunctionType
ALU = mybir.AluOpType
AX = mybir.AxisListType


class Cfg:
    def __init__(self, n_o=16384, n_u=16384, d=1024, cores=8, rowt=512):
        self.n_o, self.n_u, self.d, self.cores, self.rowt = n_o, n_u, d, cores, rowt
        self.c = 51
        self.s = n_o // cores          # labeled rows per core
        self.u = n_u // cores          # unlabeled rows per core
        self.kc = d // 128             # contraction chunks
        self.lab_tiles = self.s // rowt
        self.unl_tiles = self.u // rowt
        self.cpt = rowt // 128         # 128-row chunks per tile
        self.lab_chunks = self.s // 128
        self.chunks = self.u // 128    # unlabeled 128-row chunks
        self.trow = 128                # table row f32 elems (512B; %256B for gather)
        assert self.s % rowt == 0 and self.u % rowt == 0 and d % 128 == 0


def _bc(tile_ap, offset_ap, pattern):
    """AP on tile_ap's tensor at offset_ap's offset with a custom free pattern."""
    return bass.AP(tensor=tile_ap.tensor, offset=offset_ap.offset,
                   ap=[tile_ap.ap[0]] + pattern)


def build_bass(cfg: Cfg, use_bias: bool):
    C, TROW, KC, ROWT = cfg.c, cfg.trow, cfg.kc, cfg.rowt
    WTC = 64 + C  # Wo head starts at partition 64 (PE base-partition rule)
    nc = bacc.Bacc("TRN2", target_bir_lowering=False, debug=False,
                   num_devices=cfg.cores)

    x_h = nc.dram_tensor("x", [cfg.s + cfg.u, cfg.d], BF16, kind="ExternalInput")
    wt_h = nc.dram_tensor("wt", [cfg.d, WTC], BF16, kind="ExternalInput")
    consts_h = nc.dram_tensor("consts", [128, 3 * C], F32, kind="ExternalInput")
    labelf_h = nc.dram_tensor("labelf", [128, cfg.lab_chunks], F32,
                              kind="ExternalInput")
    gidx_h = nc.dram_tensor("gidx", [128, cfg.chunks * 40], I16,
                            kind="ExternalInput")
    biascol_h = nc.dram_tensor("biascol", [WTC, 2], F32, kind="ExternalInput")
    out_h = nc.dram_tensor("out", [1, 1], F32, kind="ExternalOutput")

    rg = [list(range(cfg.cores))]
    W5 = cfg.chunks * 5

    with tile.TileContext(nc) as tc:
        ppcm = tc.tile_pool(name="persist", bufs=1)
        pp_ = ppcm.__enter__()

        def P(shape, dtype, name):
            return pp_.tile(shape, dtype, name=name, tag=name)

        # ---- persistent/constant SBUF (loads issued from Sync, first) ----
        wt_sb = P([128, KC, WTC], BF16, "wt_sb")
        nc.sync.dma_start(
            out=wt_sb[:],
            in_=bass.AP(tensor=wt_h, offset=0,
                        ap=[[WTC, 128], [128 * WTC, KC], [1, WTC]]))
        consts_sb = P([128, 3 * C], F32, "consts_sb")
        nc.sync.dma_start(out=consts_sb[:], in_=consts_h[:])
        iota_r = consts_sb[:, 0:C]
        gm_r = consts_sb[:, C:2 * C]
        gt_r = consts_sb[:, 2 * C:3 * C]
        labelf_sb = P([128, cfg.lab_chunks], F32, "labelf_sb")
        nc.sync.dma_start(out=labelf_sb[:], in_=labelf_h[:])
        gidx_sb = P([128, cfg.chunks * 40], I16, "gidx_sb")
        nc.sync.dma_start(out=gidx_sb[:], in_=gidx_h[:])
        ident = P([128, 128], F32, "ident")
        make_identity(nc, ident[:])
        ones128 = P([128, 1], F32, "ones128")
        nc.vector.memset(ones128[:], 1.0)
        if use_bias:
            biascol_sb = P([WTC, 2], F32, "biascol_sb")
            nc.sync.dma_start(out=biascol_sb[:], in_=biascol_h[:])

        zu_all = P([128, cfg.chunks, C], F32, "zu_all")
        ohu_all = P([128, cfg.chunks, C], F32, "ohu_all")
        wbuf = P([128, 2, cfg.chunks], F32, "wbuf")
        d1buf = P([128, W5], F32, "d1buf")
        dotbuf = P([128, W5], F32, "dotbuf")
        nmbuf = P([128, W5], F32, "nmbuf")   # -max(l) per pair (stable lse)

        with tc.tile_pool(name="dramp", bufs=1, space="DRAM") as dramp:
            t_local = dramp.tile([cfg.s, TROW], BF16, name="t_local")
            t_full = dramp.tile([cfg.n_o, TROW], BF16, name="t_full",
                                addr_space="Shared")
            p_local = dramp.tile([1, 2], F32, name="p_local")
            p_full = dramp.tile([cfg.cores, 2], F32, name="p_full",
                                addr_space="Shared")

            with (
                tc.tile_pool(name="xt", bufs=cfg.lab_tiles + cfg.unl_tiles)
                    as xt_pool,
                tc.tile_pool(name="ztp", bufs=2, space="PSUM") as zt_pool,
                tc.tile_pool(name="zts", bufs=2) as zts_pool,
                tc.tile_pool(name="trp", bufs=4, space="PSUM") as tr_pool,
                tc.tile_pool(name="ppp", bufs=1, space="PSUM") as pp_pool,
                tc.tile_pool(name="lrow", bufs=3) as lrow_pool,
                tc.tile_pool(name="small", bufs=8) as small_pool,
                tc.tile_pool(name="stat", bufs=16) as stat_pool,
                tc.tile_pool(name="gp", bufs=cfg.chunks) as g_pool,
                tc.tile_pool(name="wide", bufs=2) as wide_pool,
            ):
                # ---- all transposed feat loads, issued up front ----
                # labeled tiles first (they gate the AllGather), split
                # across both HWDGE engines
                nt = cfg.lab_tiles + cfg.unl_tiles
                xts = [None] * nt
                half = cfg.lab_tiles // 2
                sync_order = (list(range(half)) +
                              list(range(cfg.lab_tiles,
                                         cfg.lab_tiles + cfg.unl_tiles // 2)))
                scal_order = (list(range(half, cfg.lab_tiles)) +
                              list(range(cfg.lab_tiles + cfg.unl_tiles // 2, nt)))
                for t in range(nt):
                    xt = xt_pool.tile([128, KC, ROWT], BF16, name="xt",
                                      tag="xt")
                    r0 = t * ROWT
                    nc.sync.dma_start_transpose(xt[:], x_h[r0:r0 + ROWT, :])
                    xts[t] = xt

                def matmul_tile(xt, m, copy_eng):
                    zt = zt_pool.tile([m, ROWT], F32, tag="zt", name="zt")
                    for k in range(KC):
                        nc.tensor.matmul(
                            zt[:], lhsT=wt_sb[:, k, 0:m],
                            rhs=xt[:, k, :], start=(k == 0), stop=(k == KC - 1))
                    zts = zts_pool.tile([m, ROWT], F32, tag="zts", name="zts")
                    if use_bias:
                        col = 0 if m == C else 1
                        if copy_eng is nc.scalar:
                            nc.scalar.add(zts[:], zt[:],
                                          biascol_sb[0:m, col:col + 1])
                        else:
                            nc.vector.tensor_scalar(
                                out=zts[:], in0=zt[:],
                                scalar1=biascol_sb[0:m, col:col + 1],
                                scalar2=None, op0=ALU.add)
                    elif copy_eng is nc.scalar:
                        nc.scalar.copy(zts[:], zt[:])
                    else:
                        nc.vector.tensor_copy(zts[:], zt[:])
                    return zts

                # ================= Phase A: labeled =================
                for t in range(cfg.lab_tiles):
                    zts = matmul_tile(xts[t], C, nc.vector)
                    for q in range(cfg.cpt):
                        g = t * cfg.cpt + q
                        tr = tr_pool.tile([128, C], F32, tag="tr", name="tr")
                        nc.tensor.transpose(tr[:], zts[0:C, q * 128:(q + 1) * 128],
                                            ident[0:C, 0:C])
                        lt = lrow_pool.tile([128, 2 * C], BF16, tag="lt",
                                            name="lt")
                        nc.vector.tensor_scalar_mul(lt[:, 0:C], tr[:], 0.7)
                        nc.vector.tensor_scalar(
                            out=lt[:, C:2 * C], in0=iota_r,
                            scalar1=labelf_sb[:, g:g + 1], scalar2=None,
                            op0=ALU.is_equal)
                        nc.sync.dma_start(
                            out=t_local[g * 128:(g + 1) * 128, 0:2 * C],
                            in_=lt[:])

                nc.gpsimd.collective_compute(
                    "AllGather", ALU.bypass, replica_groups=rg,
                    ins=[t_local[:].opt()], outs=[t_full[:].opt()])

                # ============ Phase 3a: issue all gathers early ============
                # 2 chunks (1280 rows) per dma_gather call to amortize the
                # SWDGE fixed cost; desc-gen rate is the phase-3 floor.
                call_groups = [[g] for g in range(cfg.chunks)]
                g_tiles = {}
                for grp in call_groups:
                    n_idx = 640 * len(grp)
                    gt_t = g_pool.tile([128, 5 * len(grp), TROW], BF16,
                                       tag="g", name="gt_t")
                    c0 = grp[0] * 40
                    nc.gpsimd.dma_gather(
                        out_ap=gt_t[:], in_ap=t_full[:],
                        idxs_ap=gidx_sb[:, c0:c0 + n_idx // 16],
                        num_idxs=n_idx, num_idxs_reg=n_idx, elem_size=TROW)
                    for k, g in enumerate(grp):
                        g_tiles[g] = (gt_t, k)

                # ================= Phase B: unlabeled =================
                for t in range(cfg.unl_tiles):
                    zts = matmul_tile(xts[cfg.lab_tiles + t], WTC, nc.scalar)
                    for q in range(cfg.cpt):
                        g = t * cfg.cpt + q
                        trw = tr_pool.tile([128, C], F32, tag="tr", name="trw")
                        nc.tensor.transpose(trw[:], zts[0:C, q * 128:(q + 1) * 128],
                                            ident[0:C, 0:C])
                        tro = tr_pool.tile([128, C], F32, tag="tr", name="tro")
                        nc.tensor.transpose(tro[:],
                                            zts[64:64 + C, q * 128:(q + 1) * 128],
                                            ident[64:64 + C, 64:64 + C])
                        nc.vector.tensor_scalar_mul(zu_all[:, g, :], trw[:], 0.3)
                        negm = stat_pool.tile([128, 1], F32, tag="st", name="negm")
                        nc.vector.tensor_reduce(negm[:], tro[:], axis=AX.X,
                                                op=ALU.max, negate=True)
                        ej = small_pool.tile([128, C], F32, tag="sm", name="ej")
                        svec = stat_pool.tile([128, 1], F32, tag="st", name="svec")
                        nc.scalar.activation(ej[:], tro[:], AF.Exp,
                                             bias=negm[:], scale=1.0,
                                             accum_out=svec[:])
                        # onehot(pred) = ((lo + negm) == 0), then scale 0.3
                        oh0 = small_pool.tile([128, C], F32, tag="sm", name="oh0")
                        nc.vector.tensor_scalar(
                            out=oh0[:], in0=tro[:], scalar1=negm[:],
                            scalar2=0.0, op0=ALU.add, op1=ALU.is_equal)
                        nc.vector.tensor_scalar_mul(ohu_all[:, g, :], oh0[:], 0.3)
                        gvm = stat_pool.tile([128, 1], F32, tag="st", name="gvm")
                        jm = small_pool.tile([128, C], F32, tag="sm", name="jm")
                        nc.vector.scalar_tensor_tensor(
                            out=jm[:], in0=oh0[:], scalar=1.0,
                            in1=gm_r, op0=ALU.mult, op1=ALU.mult,
                            accum_out=gvm[:])
                        gvt = stat_pool.tile([128, 1], F32, tag="st", name="gvt")
                        jt = small_pool.tile([128, C], F32, tag="sm", name="jt")
                        nc.vector.scalar_tensor_tensor(
                            out=jt[:], in0=oh0[:], scalar=1.0,
                            in1=gt_r, op0=ALU.mult, op1=ALU.mult,
                            accum_out=gvt[:])
                        nc.vector.scalar_tensor_tensor(
                            out=wbuf[:, 0, g:g + 1], in0=svec[:], scalar=2.0,
                            in1=gvm[:], op0=ALU.is_lt, op1=ALU.mult)
                        nc.vector.scalar_tensor_tensor(
                            out=wbuf[:, 1, g:g + 1], in0=svec[:],
                            scalar=float(1.0 / 0.3), in1=gvt[:],
                            op0=ALU.is_lt, op1=ALU.mult)

                # ================= Phase 3b: pair CE =================
                for g in range(cfg.chunks):
                    gt_full, koff = g_tiles[g]
                    gt_t = gt_full[:, koff * 5:(koff + 1) * 5, :]
                    g5 = g * 5
                    zub = _bc(zu_all[:], zu_all[:, g, :], [[0, 5], [1, C]])
                    ohb = _bc(ohu_all[:], ohu_all[:, g, :], [[0, 5], [1, C]])
                    lp = wide_pool.tile([128, 5, C], F32, tag="lp", name="lp")
                    nc.vector.tensor_tensor(out=lp[:], in0=gt_t[:, :, 0:C],
                                            in1=zub, op=ALU.add)
                    nc.vector.tensor_reduce(nmbuf[:, g5:g5 + 5], lp[:],
                                            axis=AX.X, op=ALU.max, negate=True)
                    lps = wide_pool.tile([128, 5, C], F32, tag="lps", name="lps")
                    nc.vector.tensor_tensor(
                        out=lps[:], in0=lp[:],
                        in1=_bc(nmbuf[:], nmbuf[:, g5:g5 + 5], [[1, 5], [0, C]]),
                        op=ALU.add)
                    ew = wide_pool.tile([128, 5, C], F32, tag="ew", name="ew")
                    nc.scalar.activation(ew[:], lps[:], AF.Exp)
                    nc.vector.tensor_reduce(d1buf[:, g5:g5 + 5], ew[:],
                                            axis=AX.X, op=ALU.add)
                    yw = wide_pool.tile([128, 5, C], F32, tag="yw", name="yw")
                    nc.vector.scalar_tensor_tensor(
                        out=yw[:], in0=gt_t[:, :, C:2 * C], scalar=0.7,
                        in1=ohb, op0=ALU.mult, op1=ALU.add)
                    pw = wide_pool.tile([128, 5, C], F32, tag="pw", name="pw")
                    nc.vector.tensor_tensor(out=pw[:], in0=lp[:], in1=yw[:],
                                            op=ALU.mult)
                    nc.vector.tensor_reduce(dotbuf[:, g5:g5 + 5], pw[:],
                                            axis=AX.X, op=ALU.add)

                # ================= Final reduction =================
                lse = P([128, W5], F32, "lse")
                nc.scalar.activation(lse[:], d1buf[:], AF.Ln)
                ce = P([128, W5], F32, "ce")
                nc.vector.tensor_tensor(out=ce[:], in0=lse[:], in1=nmbuf[:],
                                        op=ALU.subtract)   # lse + m
                nc.vector.tensor_tensor(out=ce[:], in0=ce[:], in1=dotbuf[:],
                                        op=ALU.subtract)
                accw = P([128, 2], F32, "accw")
                amid = P([128, 1], F32, "amid")
                jA = P([128, cfg.chunks, 2], F32, "jA")
                ce3 = bass.AP(tensor=ce[:].tensor, offset=ce[:].offset,
                              ap=[ce[:].ap[0], [5, cfg.chunks], [1, 2]])
                wA = _bc(wbuf[:], wbuf[:, 0, :], [[1, cfg.chunks], [0, 2]])
                nc.vector.scalar_tensor_tensor(
                    out=jA[:], in0=ce3, scalar=1.0, in1=wA,
                    op0=ALU.mult, op1=ALU.mult, accum_out=amid[:])
                atail = P([128, 1], F32, "atail")
                jB = P([128, cfg.chunks, 3], F32, "jB")
                ce2 = bass.AP(tensor=ce[:].tensor, offset=ce[:, 2:3].offset,
                              ap=[ce[:].ap[0], [5, cfg.chunks], [1, 3]])
                wB = _bc(wbuf[:], wbuf[:, 1, :], [[1, cfg.chunks], [0, 3]])
                nc.vector.scalar_tensor_tensor(
                    out=jB[:], in0=ce2, scalar=1.0, in1=wB,
                    op0=ALU.mult, op1=ALU.mult, accum_out=atail[:])
                nc.vector.tensor_tensor(out=accw[:, 0:1], in0=amid[:],
                                        in1=atail[:], op=ALU.add)
                # w_sum = 2*sum(midw) + 3*sum(tailw)
                smid = P([128, 1], F32, "smid")
                nc.vector.tensor_reduce(smid[:], wbuf[:, 0, :], axis=AX.X,
                                        op=ALU.add)
                stail = P([128, 1], F32, "stail")
                nc.vector.tensor_reduce(stail[:], wbuf[:, 1, :], axis=AX.X,
                                        op=ALU.add)
                st3 = P([128, 1], F32, "st3")
                nc.vector.tensor_scalar_mul(st3[:], stail[:], 3.0)
                nc.vector.scalar_tensor_tensor(
                    out=accw[:, 1:2], in0=smid[:], scalar=2.0, in1=st3[:],
                    op0=ALU.mult, op1=ALU.add)
                pp = pp_pool.tile([1, 2], F32, name="pp")
                nc.tensor.matmul(pp[:], lhsT=ones128[:], rhs=accw[:],
                                 start=True, stop=True)
                ppsb = P([1, 2], F32, "ppsb")
                nc.vector.tensor_copy(ppsb[:], pp[:])
                nc.sync.dma_start(out=p_local[:], in_=ppsb[:])
                nc.gpsimd.collective_compute(
                    "AllGather", ALU.bypass, replica_groups=rg,
                    ins=[p_local[:].opt()], outs=[p_full[:].opt()])
                pf = P([1, 2 * cfg.cores], F32, "pf")
                nc.sync.dma_start(
                    out=pf[:],
                    in_=bass.AP(tensor=p_full[:].tensor, offset=p_full[:].offset,
                                ap=[[0, 1], [1, 2 * cfg.cores]]))
                red = P([1, 2], F32, "red")
                nc.vector.tensor_reduce(
                    red[:],
                    bass.AP(tensor=pf[:].tensor, offset=pf[:].offset,
                            ap=[pf[:].ap[0], [1, 2], [2, cfg.cores]]),
                    axis=AX.X, op=ALU.add)
                cmax = P([1, 1], F32, "cmax")
                nc.vector.tensor_scalar_max(cmax[:], red[:, 1:2], 1.0)
                rec = P([1, 1], F32, "rec")
                nc.vector.reciprocal(rec[:], cmax[:])
                fin = P([1, 1], F32, "fin")
                nc.vector.tensor_tensor(out=fin[:], in0=red[:, 0:1], in1=rec[:],
                                        op=ALU.mult)
                nc.sync.dma_start(out=out_h[:], in_=fin[:])

        ppcm.__exit__(None, None, None)

    nc.compile()
    return nc


def make_in_maps(cfg: Cfg, feat, label, W_o, b_o, W, b, gm, gt, idx_m, idx_t):
    """Host-side shard/prep. Returns (in_maps, use_bias)."""
    n_o, C = cfg.n_o, cfg.c
    feat = np.ascontiguousarray(np.asarray(feat, np.float32))
    label = np.asarray(label).astype(np.int64)
    W_o = np.asarray(W_o, np.float32)
    W = np.asarray(W, np.float32)
    b_o = np.asarray(b_o, np.float32)
    b = np.asarray(b, np.float32)
    gm = np.asarray(gm).astype(np.float32)
    gt = np.asarray(gt).astype(np.float32)
    idxs = np.concatenate([np.asarray(idx_m), np.asarray(idx_t)], 0).astype(np.int64)

    use_bias = bool(np.any(b) or np.any(b_o))
    feat_bf = feat.astype(ml_dtypes.bfloat16)
    wt = np.zeros((cfg.d, 64 + C), np.float32)
    wt[:, 0:C] = W.T
    wt[:, 64:64 + C] = W_o.T
    wt = np.ascontiguousarray(wt.astype(ml_dtypes.bfloat16))
    consts = np.concatenate([
        np.tile(np.arange(C, dtype=np.float32), (128, 1)),
        np.tile(gm, (128, 1)),
        np.tile(gt, (128, 1)),
    ], axis=1)
    consts = np.ascontiguousarray(consts)
    biascol = np.zeros((64 + C, 2), np.float32)
    biascol[0:C, 0] = b / 0.7
    biascol[64:64 + C, 1] = b_o
    label_o = label[:n_o].astype(np.float32)

    in_maps = []
    for r in range(cfg.cores):
        lab0, unl0 = cfg.s * r, n_o + cfg.u * r
        x = np.concatenate([feat_bf[lab0:lab0 + cfg.s],
                            feat_bf[unl0:unl0 + cfg.u]], axis=0)
        labelf = label_o[lab0:lab0 + cfg.s].reshape(cfg.lab_chunks, 128).T
        gcols = []
        for a in range(0, cfg.chunks, 1):
            grp = [a]
            flats = []
            for g in grp:
                rows = cfg.u * r + g * 128 + np.arange(128)
                flats.append(idxs[:, rows].reshape(-1))   # [5*128] c-major
            flat = np.concatenate(flats)                  # [640*len(grp)]
            a16 = flat.reshape(-1, 16).T                  # [16, 40*len]
            gcols.append(np.tile(a16, (8, 1)))
        gidx = np.concatenate(gcols, axis=1).astype(np.int16)
        in_maps.append(dict(
            x=np.ascontiguousarray(x),
            wt=wt,
            consts=consts,
            labelf=np.ascontiguousarray(labelf.astype(np.float32)),
            gidx=np.ascontiguousarray(gidx),
            biascol=biascol,
        ))
    return in_maps, use_bias


_CACHE = {}


def _get_nc(cfg: Cfg, use_bias: bool):
    key = (cfg.n_o, cfg.n_u, cfg.d, cfg.cores, cfg.rowt, use_bias)
    if key not in _CACHE:
        _CACHE[key] = build_bass(cfg, use_bias)
    return _CACHE[key]


def _install_ntff_shim():
    """This image's antenv lacks axon_hooks; recreate it so trace=True works."""
    import sys
    import types
    try:
        from antenv.axon_hooks import get_axon_ntff_profile_hook  # noqa: F401
        return
    except ImportError:
        pass
    try:
        import antenv
        from trn_agent_boot.trn_boot import _ntff_profile_via_ctypes
        h = _ntff_profile_via_ctypes("/opt/axon/libaxon_pjrt.so")
        mod = types.ModuleType("antenv.axon_hooks")
        mod.get_axon_ntff_profile_hook = lambda: h
        mod.set_axon_ntff_profile_hook = lambda hook: None
        sys.modules["antenv.axon_hooks"] = mod
        antenv.axon_hooks = mod
    except Exception:
        pass


def kernel(feat, label, W_o, b_o, W, b, group_mid_mask, group_tail_mask,
           idx_m, idx_t, _trace=False):
    if _trace:
        _install_ntff_shim()
    n_u = int(np.asarray(idx_m).shape[1])
    n_o = int(np.asarray(feat).shape[0]) - n_u
    cfg = Cfg(n_o=n_o, n_u=n_u, d=int(np.asarray(feat).shape[1]))
    in_maps, use_bias = make_in_maps(cfg, feat, label, W_o, b_o, W, b,
                                     group_mid_mask, group_tail_mask,
                                     idx_m, idx_t)
    nc = _get_nc(cfg, use_bias)
    res = run_bass_kernel_spmd(nc, in_maps, core_ids=list(range(cfg.cores)),
                               trace=_trace)
    out = np.float32(res.results[0]["out"].reshape(-1)[0])
    if _trace:
        return out, res
    return out
